# revision 2
# baseline (speedup 1.0000x reference)
"""GCN layer kernel for Trainium2 (8 NeuronCores, SPMD).

Computes relu(A_hat @ (H @ W) + b) as relu((A_hat @ H) @ W + b)
(segment-sum commutes with the dense feature transform).

Strategy:
  - Shard destination nodes across 8 cores (12500 rows each); edges are
    pre-partitioned by destination row on the host.
  - Per core, destinations are processed in 98 windows of 128 rows. Edges
    of a window are split into 4 source buckets of 25000 nodes (int16
    gather-index limit) and padded to chunks of 128 edges.
  - Chunk structure (chunks per window x bucket) is the max over cores so
    a single SPMD program serves all 8 cores.
  - Per chunk: dma_gather 128 source rows (HBM -> SBUF, 4 SWDGE queues),
    build a one-hot-times-val matrix S^T[e,d] = val_e * (dest_e == d) with
    one fused DVE tensor_scalar, and accumulate agg^T[f,d] += G^T S^T on
    the tensor engine (PSUM, K=128 edges).
  - Per window: agg^T (PSUM) -> SBUF, final matmul with W, add bias, ReLU,
    DMA out.
"""

import os

import numpy as np

N = 100000
E = 1600000
F = 64
NCORES = 8
ND = N // NCORES          # 12500 destination rows per core
WSZ = 128                 # window = 128 destination rows
NW = (ND + WSZ - 1) // WSZ  # 98 windows (last has 84 rows)
NBUCK = 4                 # source buckets (int16 index limit)
BSZ = N // NBUCK          # 25000 source rows per bucket
GRP = 8                   # windows per pipeline group
NQ = 4                    # SWDGE gather queues

_cache = {}


def _preprocess(edge_row, edge_col, edge_vals):
    """Return (layout, per-core host arrays)."""
    core = edge_row // ND
    r_local = edge_row - core * ND
    w = r_local // WSZ
    r_in_w = r_local - w * WSZ
    b = edge_col // BSZ
    col_local = (edge_col - b * BSZ).astype(np.int16)

    key = ((core.astype(np.int64) * NW + w) * NBUCK + b)
    counts = np.bincount(key, minlength=NCORES * NW * NBUCK).reshape(
        NCORES, NW, NBUCK
    )
    m = (counts + 127) // 128
    m = m.max(axis=0)                      # [NW, NBUCK] chunks per (w, b)
    m[:, 0] = np.maximum(m[:, 0], 1)       # every window has >= 1 chunk

    # global chunk order: (group, bucket, window-in-group, chunk)
    groups = [list(range(g, min(g + GRP, NW))) for g in range(0, NW, GRP)]
    seg_start = np.zeros((NW, NBUCK), dtype=np.int64)  # chunk index of (w,b)
    group_info = []  # per group: (windows, chunk_start, ncols, [(b, call_cols)])
    ch = 0
    for ws in groups:
        g_start = ch
        calls = []
        for bb in range(NBUCK):
            c0 = ch
            for ww in ws:
                seg_start[ww, bb] = ch
                ch += m[ww, bb]
            calls.append((bb, ch - c0))
        group_info.append((ws, g_start, ch - g_start, calls))
    tch = ch                                # total chunks per core

    # per-core padded streams
    order = np.argsort(key, kind="stable")
    # rank of each edge within its (core, w, b) segment
    seg_sizes = counts.reshape(-1)
    seg_off = np.zeros_like(seg_sizes)
    np.cumsum(seg_sizes[:-1], out=seg_off[1:])
    rank_sorted = np.arange(E, dtype=np.int64) - np.repeat(seg_off, seg_sizes)
    rank = np.empty(E, dtype=np.int64)
    rank[order] = rank_sorted

    # padded position of each edge inside its core's chunk stream
    pos = seg_start[w, b] * 128 + rank
    pe = tch * 128

    idx_all = np.zeros((NCORES, pe), dtype=np.int16)
    rr_all = np.zeros((NCORES, 128, tch), dtype=np.float32)
    vv_all = np.zeros((NCORES, 128, tch), dtype=np.float32)
    idx_all[core, pos] = col_local
    rr_all[core, pos % 128, pos // 128] = r_in_w.astype(np.float32)
    vv_all[core, pos % 128, pos // 128] = edge_vals

    # wrap gather indices per call: within a call's slice of length L the
    # SBUF tile is [128, L//16] with logical idx i at [i%16, i//16],
    # replicated 8x across partition groups.
    gidx = np.zeros((NCORES, 128, pe // 16), dtype=np.int16)
    for ws, g_start, ncols, calls in group_info:
        o = g_start
        for bb, ccols in calls:
            L = ccols * 128
            s = o * 128
            sl = idx_all[:, s:s + L].reshape(NCORES, L // 16, 16)
            wrapped = np.swapaxes(sl, 1, 2)          # [NCORES, 16, L//16]
            gidx[:, :, s // 16:(s + L) // 16] = np.tile(wrapped, (1, 8, 1))
            o += ccols

    layout = (tuple(map(tuple, m.tolist())), tuple(
        (tuple(ws), g_start, ncols, tuple(calls))
        for ws, g_start, ncols, calls in group_info
    ), tch)
    return layout, gidx, rr_all, vv_all


def _install_trace_hook():
    import sys
    import types

    if "antenv.axon_hooks" in sys.modules:
        return
    mod = types.ModuleType("antenv.axon_hooks")

    def set_hook(h):
        mod._hook = h

    def get_hook():
        return getattr(mod, "_hook", None)

    mod.set_axon_ntff_profile_hook = set_hook
    mod.get_axon_ntff_profile_hook = get_hook
    sys.modules["antenv.axon_hooks"] = mod
    try:
        from trn_agent_boot.trn_boot import _ntff_profile_via_ctypes

        mod._hook = _ntff_profile_via_ctypes("/opt/axon/libaxon_pjrt.so")
    except Exception:
        mod._hook = None


def _build(layout):
    import concourse.mybir as mybir
    import concourse.tile as tile
    from concourse import bacc

    m_t, group_info, tch = layout
    m = np.array(m_t, dtype=np.int64)
    pe = tch * 128

    nc = bacc.Bacc("TRN2", target_bir_lowering=False, debug=False,
                   num_devices=NCORES, num_swdge_queues=NQ)
    f32 = mybir.dt.float32
    src = nc.dram_tensor("src", [N, F], f32, kind="ExternalInput").ap()
    gidx = nc.dram_tensor("gidx", [128, pe // 16], mybir.dt.int16,
                          kind="ExternalInput").ap()
    rr = nc.dram_tensor("rr", [128, tch], f32, kind="ExternalInput").ap()
    vv = nc.dram_tensor("vv", [128, tch], f32, kind="ExternalInput").ap()
    iota = nc.dram_tensor("iota", [128, 128], f32, kind="ExternalInput").ap()
    wmat = nc.dram_tensor("wmat", [F, F], f32, kind="ExternalInput").ap()
    bias = nc.dram_tensor("bias", [128, F], f32, kind="ExternalInput").ap()
    out = nc.dram_tensor("out", [ND, F], f32, kind="ExternalOutput").ap()

    max_cols = max(gi[2] for gi in group_info)

    with tile.TileContext(nc) as tc:
        with (
            tc.tile_pool(name="const", bufs=1) as constp,
            tc.tile_pool(name="meta", bufs=2) as metap,
            tc.tile_pool(name="gat", bufs=2) as gatp,
            tc.tile_pool(name="st", bufs=6) as stp,
            tc.tile_pool(name="ps1", bufs=3, space="PSUM") as ps1p,
            tc.tile_pool(name="ps2", bufs=2, space="PSUM") as ps2p,
            tc.tile_pool(name="agg", bufs=3) as aggp,
            tc.tile_pool(name="ob", bufs=4) as obp,
        ):
            iota_t = constp.tile([128, 128], f32)
            nc.sync.dma_start(out=iota_t[:], in_=iota[:])
            w_t = constp.tile([F, F], f32)
            nc.sync.dma_start(out=w_t[:], in_=wmat[:])
            bias_t = constp.tile([128, F], f32)
            nc.sync.dma_start(out=bias_t[:], in_=bias[:])

            for ws, g_start, ncols, calls in group_info:
                # stage group metadata (indices + one-hot params)
                idx_t = metap.tile([128, max_cols * 8], mybir.dt.int16,
                                   tag="idx")
                nc.sync.dma_start(
                    out=idx_t[:, :ncols * 8],
                    in_=gidx[:, g_start * 8:(g_start + ncols) * 8])
                rr_t = metap.tile([128, max_cols], f32, tag="rr")
                nc.sync.dma_start(out=rr_t[:, :ncols],
                                  in_=rr[:, g_start:g_start + ncols])
                vv_t = metap.tile([128, max_cols], f32, tag="vv")
                nc.sync.dma_start(out=vv_t[:, :ncols],
                                  in_=vv[:, g_start:g_start + ncols])

                g_t = gatp.tile([128, max_cols, F], f32, tag="g")
                o = 0
                for bb, ccols in calls:
                    if ccols == 0:
                        continue
                    nidx = ccols * 128
                    nc.gpsimd.dma_gather(
                        g_t[:, o:o + ccols, :],
                        src[bb * BSZ:(bb + 1) * BSZ, :],
                        idx_t[:, o * 8:o * 8 + nidx // 16],
                        num_idxs=nidx, num_idxs_reg=nidx, elem_size=F,
                        single_packet=False, queue_num=bb % NQ,
                    )
                    o += ccols

                for ww in ws:
                    chunks = []
                    o = 0
                    for bb, ccols in calls:
                        lo = sum(int(m[w2, bb]) for w2 in ws if w2 < ww)
                        chunks.extend(range(o + lo, o + lo + int(m[ww, bb])))
                        o += ccols
                    ps1 = ps1p.tile([F, 128], f32, space="PSUM")
                    for j, ci in enumerate(chunks):
                        st = stp.tile([128, 128], f32)
                        nc.vector.tensor_scalar(
                            out=st[:], in0=iota_t[:],
                            scalar1=rr_t[:, ci:ci + 1],
                            scalar2=vv_t[:, ci:ci + 1],
                            op0=mybir.AluOpType.is_equal,
                            op1=mybir.AluOpType.mult,
                        )
                        nc.tensor.matmul(
                            out=ps1[:], lhsT=g_t[:, ci, :], rhs=st[:],
                            start=(j == 0), stop=(j == len(chunks) - 1),
                        )
                    aggT = aggp.tile([F, 128], f32)
                    nc.vector.tensor_copy(out=aggT[:], in_=ps1[:])
                    ps2 = ps2p.tile([128, F], f32, space="PSUM")
                    nc.tensor.matmul(out=ps2[:], lhsT=aggT[:], rhs=w_t[:],
                                     start=True, stop=True)
                    ob = obp.tile([128, F], f32, tag="ob")
                    nc.vector.tensor_tensor(out=ob[:], in0=ps2[:],
                                            in1=bias_t[:],
                                            op=mybir.AluOpType.add)
                    ob2 = obp.tile([128, F], f32, tag="ob2")
                    nc.scalar.activation(ob2[:], ob[:],
                                         mybir.ActivationFunctionType.Relu)
                    rows = min(WSZ, ND - ww * WSZ)
                    nc.scalar.dma_start(out=out[ww * WSZ:ww * WSZ + rows, :],
                                        in_=ob2[:rows, :])
    nc.compile()
    return nc


def kernel(node_features, edge_row, edge_col, edge_vals, kernel, bias):
    from concourse.bass_utils import run_bass_kernel_spmd

    trace = os.environ.get("GCN_TRACE", "") == "1"
    if trace:
        _install_trace_hook()

    node_features = np.ascontiguousarray(node_features, dtype=np.float32)
    edge_row = np.asarray(edge_row, dtype=np.int64)
    edge_col = np.asarray(edge_col, dtype=np.int64)
    edge_vals = np.ascontiguousarray(edge_vals, dtype=np.float32)
    wmat = np.ascontiguousarray(kernel, dtype=np.float32)
    bias = np.asarray(bias, dtype=np.float32)

    layout, gidx, rr_all, vv_all = _preprocess(edge_row, edge_col, edge_vals)

    key = hash(repr(layout))
    if key not in _cache:
        _cache[key] = _build(layout)
    nc = _cache[key]

    iota_v = np.ascontiguousarray(
        np.tile(np.arange(128, dtype=np.float32)[None, :], (128, 1)))
    bias_v = np.ascontiguousarray(np.tile(bias[None, :], (128, 1)))
    in_maps = []
    for c in range(NCORES):
        in_maps.append({
            "src": node_features,
            "gidx": np.ascontiguousarray(gidx[c]),
            "rr": np.ascontiguousarray(rr_all[c]),
            "vv": np.ascontiguousarray(vv_all[c]),
            "iota": iota_v,
            "wmat": wmat,
            "bias": bias_v,
        })
    res = run_bass_kernel_spmd(nc, in_maps, core_ids=list(range(NCORES)),
                               trace=trace)
    if trace and res.exec_time_ns is not None:
        print(f"HW exec time: {res.exec_time_ns} ns")
        globals()["_last_exec_ns"] = res.exec_time_ns
        globals()["_last_trace"] = (res.instructions_and_trace or (None, None))[1]
    return np.concatenate([res.results[c]["out"] for c in range(NCORES)],
                          axis=0)
